# revision 22
# baseline (speedup 1.0000x reference)
"""GAT-style attentive layer on 8 TRN2 NeuronCores — fp8 DoubleRow version.

Math (per reference):
    Wh  = input                      [N, D]   (N=8192, D=512)
    Wh1 = Wh @ a[:D]; Wh2 = Wh @ a[D:]
    e   = leaky_relu(Wh1 + Wh2.T, 0.01)
    att = softmax(where(adj>0, e, -9e15), axis=1)
    out = att @ Wh

Sharding: row-shard the N x N attention across 8 cores (1024 rows each).

Key techniques vs the fp32r baseline (timings per the TimelineSim model):
  * Attention matmul in fp8 DoubleRow mode: lhsT = pm tile broadcast
    (stride-0) across the two DR slots, rhs = (hi, lo) fp8 split of Wh, so
    each MM contracts one j-tile at hi+lo precision in 256 cycles (4x the
    fp32r baseline's throughput per output element at matched accuracy).
  * exp act table patched: x>=0 -> exp(x)/4; -16<x<0 -> exp(0.01x)/4 (fused
    leaky-relu); x<=-16 -> 0.  The /4 keeps exp inside TRN fp8e4's +-240
    range; softmax normalization cancels it exactly.  The zero region makes
    the {0,-30} additive adjacency encoding a free mask for route C.
  * Scores are produced by three routes, balancing ACT / PE / DVE / Pool:
      A: ACT table op (in = broadcast Wh1, bias = Wh2 col) -> T staging,
         then one quad-wide is_ge-gate stt -> masked fp8 pm.
      B: exp(lrelu(s)) = max(exp(s), exp(.01 s)) as outer products of
         true-exp vectors via a custom fused DVE op, then the same gate.
      C: PE computes s + mask additively in PSUM (rank-2 [Wh2row; ones]
         matmul + identity x adj(0/-30) matmul); ACT reads PSUM and writes
         masked fp8 pm directly - no DVE/Pool gate at all.
  * Wh2 = x @ a2 in ONE stt per j-tile over [hi(512) | lo-top(256)] packed
    columns: features are pre-sorted by |a2| on the host so the top-|a2|
    half of the lo residual (7/8 of its variance) is contiguous.
  * Row sums ride the same stride-0 lhsT with a (1,0) fp8 ones pair.
"""

import os
import numpy as np
import ml_dtypes

import concourse.bass as bass
import concourse.mybir as mybir
import concourse.tile as tile
from concourse import bacc
from concourse.bass_utils import run_bass_kernel_spmd

N = 8192
D = 512
NCORES = 8
ROWS = N // NCORES   # 1024 output rows per core
P = 128
NJT = 64             # j-tiles per core pass
NQ = 16              # j-quads per ic pass
IC_W = 512
NIC = ROWS // IC_W   # 2 i-chunks
ITPC = IC_W // P     # 4 i-subtiles per chunk

TBL_SCALE = 0.25
IMM2 = float(4.0 * np.exp(-8.0))
LO_W = 256           # wh2 lo-correction width (top-|a2| columns)
W2_W = D + LO_W      # combined wh2 reduction width

# Route per quad, per ic: 'A' ACT-table, 'B' custom-DVE, 'C' PE-additive.
ROUTES = [
    ['A'] * 16,
    ['A'] * 16,
]
# engine for wh2 reduction / gate per quad: 'v' DVE, 'g' Pool
WH2_ENG = ['v'] * 16   # gpsimd stt+accum_out fails walrus codegen
GATE_ENG = ['v'] * 16  # walrus: Pool rejects TensorScalarPtr entirely

AF = mybir.ActivationFunctionType
ALU = mybir.AluOpType
dt = mybir.dt
F32 = dt.float32
F32R = dt.float32r
BF16 = dt.bfloat16
FP8 = dt.float8e4
DR = mybir.MatmulPerfMode.DoubleRow


def register_max_outer():
    """Custom fused DVE op: out = max(in0*s0, in1*s1) * imm2."""
    import concourse.dve_ops as dops
    from concourse.dve_spec import Spec, Src0, Src1, C0, C1, C2, maxx, lower
    if "MAX_OUTER_ANT" in dops._SUB_OPCODE_FOR_NAME:
        return next(op for op in dops.OPS if op.name == "MAX_OUTER_ANT")
    spec = Spec(
        body=maxx(Src0 * C0, Src1 * C1) * C2,
        reference=lambda in0, in1, s0, s1, imm2: np.maximum(
            in0 * s0, in1 * s1) * imm2,
    )
    from concourse.dve_table_gen import DveOpSpec
    shas = {}
    for ver in ("v3", "v4"):
        tmp = DveOpSpec(name="MAX_OUTER_ANT", opcode=0,
                        uops=lower(spec, ver=ver), rd1_en=True)
        shas[ver] = tmp.sha(ver)
    op = dops.DveOp("MAX_OUTER_ANT", spec, subdim=False, uops_sha=shas)
    dops.OPS.append(op)
    dops._SUB_OPCODE_FOR_NAME[op.name] = dops._CUSTOM_DVE_ROW_BASE + len(dops.OPS) - 1
    dops.CUSTOM_DVE_SPECS[op.name] = op.spec
    return op


def _make_fused_act_root() -> str:
    """Patch the exp table: x>=0 -> exp(x)/4; -16<x<0 -> exp(0.01x)/4;
    x<=-16 -> 0."""
    import json
    import shutil
    import tempfile

    from neuronxcc.driver.Job import Job
    from neuronxcc.driver.jobs.support.FindActInfo import findActInfoFile

    src_root = os.path.dirname(findActInfoFile(Job.getPackageDir(), "gen3"))
    dst = tempfile.mkdtemp(prefix="act_root_fused_")
    for f in os.listdir(src_root):
        shutil.copy(os.path.join(src_root, f), os.path.join(dst, f))
    info = json.load(open(os.path.join(dst, "act_info.json")))
    for s in info["act_func_sets"]:
        if "exp" not in s["act"]:
            continue
        prof = json.load(open(os.path.join(dst, s["profile_json"])))
        order = sorted(prof["func_to_bkt_start_idx"].items(), key=lambda kv: kv[1])
        idx = [i for i, (k, _) in enumerate(order) if k == "exp"][0]
        lo = order[idx][1]
        hi = order[idx + 1][1] if idx + 1 < len(order) else prof["bkt_entry_cnt"]
        path = os.path.join(dst, s["bkt_bin"])
        bkt = np.fromfile(path, dtype=np.float32).reshape(-1, 8).copy()
        for b in range(lo, hi):
            d0, d1, _, _, x0 = bkt[b, :5]
            if not (d0 > 0 and abs(d1 - d0) <= 1e-3 * d0):
                continue  # saturation buckets (inf / 0)
            if x0 >= 0:
                bkt[b, 0:4] *= np.float32(TBL_SCALE)
            elif x0 <= -16.0:
                bkt[b, 0:4] = 0.0
            else:
                g = np.float32(np.exp(0.01 * np.float64(x0)) * TBL_SCALE)
                bkt[b, 0] = g
                bkt[b, 1] = np.float32(0.01 * g)
                bkt[b, 2] = np.float32(0.0)
                bkt[b, 3] = np.float32(0.0)
        bkt.tofile(path)
    return os.path.join(dst, "act_info.json")


def _build_kernel(nc: bass.Bass, tc: tile.TileContext,
                  adjT: bass.AP, xw2: bass.AP, xloc: bass.AP, a: bass.AP,
                  eye8: bass.AP, out: bass.AP, ctx, max_outer):
    pool_const = ctx.enter_context(tc.tile_pool(name="const", bufs=1))
    pool_wh = ctx.enter_context(tc.tile_pool(name="wh", bufs=1))
    pool_adj = ctx.enter_context(tc.tile_pool(name="adj", bufs=6))
    pool_pm = ctx.enter_context(tc.tile_pool(name="pm", bufs=6))
    pool_tq = ctx.enter_context(tc.tile_pool(name="tq", bufs=3))
    pool_outs = ctx.enter_context(tc.tile_pool(name="outs", bufs=1))
    pool_small = ctx.enter_context(tc.tile_pool(name="small", bufs=1))
    pool_psum = ctx.enter_context(tc.tile_pool(name="psum", bufs=1, space="PSUM"))
    pool_psc = ctx.enter_context(tc.tile_pool(name="psc", bufs=3, space="PSUM"))
    pool_dram = ctx.enter_context(tc.tile_pool(name="dram", bufs=1, space="DRAM"))

    # ---- constants --------------------------------------------------------
    abc = pool_const.tile([P, D + W2_W], F32)
    nc.sync.dma_start(abc, a)
    bcast_a1 = abc[:, 0:D]
    bcast_a2c = abc[:, D:D + W2_W]  # [a2 | a2-top] for the fused wh2 stt

    warm = pool_const.tile([1, 2], F32)
    nc.vector.memset(warm, 0.0)
    nc.scalar.activation(warm, warm, AF.Exp)

    bias4 = pool_const.tile([P, 1], F32)
    nc.vector.memset(bias4, 4.0)

    ones_hl = pool_const.tile([P, 2, 16], FP8)
    nc.vector.memset(ones_hl[:, 0:1, :], 1.0)
    nc.vector.memset(ones_hl[:, 1:2, :], 0.0)

    eyeq = pool_const.tile([P, P], FP8)
    nc.sync.dma_start(eyeq, eye8)

    # ---- Wh hi/lo quads (DMAs emitted interleaved below) ------------------
    whq2 = [pool_wh.tile([P, 8, 2, D], FP8, tag=f"whq{g}", name=f"whq{g}")
            for g in range(8)]

    def dma_whq2(g):
        nc.sync.dma_start(
            whq2[g], xw2[bass.ds(g * 8 * P, 8 * P), :, :].rearrange(
                "(q p) h d -> p q h d", p=P))

    def wh_pair(jt):  # rhs [128, 2, D] = (hi, lo) of j-tile jt
        g, q = divmod(jt, 8)
        return whq2[g][:, q, :, :]

    # ---- wh2 = x @ a2 (one fused hi+lo-top stt per j-tile) ----------------
    wh2q = [pool_small.tile([P, 4], F32, tag=f"w2t{m}", name=f"w2t{m}")
            for m in range(NQ)]
    e2q = {}

    def emit_wh2(m):
        eng = nc.vector if WH2_ENG[m] == 'v' else nc.gpsimd
        for q in range(4):
            jt = 4 * m + q
            g, q8 = divmod(jt, 8)
            src = whq2[g][:, q8, :, :].rearrange("p h d -> p (h d)")[:, 0:W2_W]
            scr = pool_small.tile([P, W2_W], F32, tag="w2s", name="w2s", bufs=3)
            eng.scalar_tensor_tensor(
                out=scr, in0=src, scalar=0.0, in1=bcast_a2c,
                op0=ALU.add, op1=ALU.mult,
                accum_out=wh2q[m][:, q:q + 1])

    # ---- Wh1 rows (bf16 local) + broadcasts + true-exp rows ---------------
    wh1_rows = [pool_const.tile([1, IC_W], F32, tag=f"wh1r{h}", name=f"wh1r{h}")
                for h in range(NIC)]
    bcast_wh1h = [pool_const.tile([P, IC_W], F32, tag=f"bwh1{h}", name=f"bwh1{h}")
                  for h in range(NIC)]
    bE1h = [pool_const.tile([P, IC_W], F32, tag=f"bE1{h}", name=f"bE1{h}")
            for h in range(NIC)]
    bF1h = [pool_const.tile([P, IC_W], F32, tag=f"bF1{h}", name=f"bF1{h}")
            for h in range(NIC)]
    one_wh1 = pool_const.tile([2, ROWS], F32)
    nc.vector.memset(one_wh1[0:1, :], 1.0)
    one_wh1R = pool_const.tile([2, ROWS], F32R)

    for h in range(NIC):
        wh1_half = pool_small.tile([P, ITPC], F32, tag=f"wh1h{h}", name=f"wh1h{h}")
        for q in range(4):
            xlt = pool_small.tile([P, D], BF16, tag="xlt", name="xlt", bufs=3)
            nc.sync.dma_start(xlt, xloc[bass.ds((h * 4 + q) * P, P), :])
            scr = pool_small.tile([P, D], F32, tag="v_scr", name="v_scr", bufs=2)
            nc.vector.scalar_tensor_tensor(
                out=scr, in0=xlt, scalar=0.0, in1=bcast_a1,
                op0=ALU.add, op1=ALU.mult,
                accum_out=wh1_half[:, q:q + 1])
        scr_d = pool_dram.tile([IC_W], F32, tag=f"wh1d{h}", name=f"wh1d{h}")
        nc.sync.dma_start(scr_d.rearrange("(t p) -> p t", p=P), wh1_half)
        nc.sync.dma_start(wh1_rows[h], scr_d.rearrange("(o n) -> o n", o=1))
        nc.sync.dma_start(one_wh1[1:2, bass.ds(h * IC_W, IC_W)],
                          scr_d.rearrange("(o n) -> o n", o=1))
        nc.vector.tensor_copy(out=one_wh1R[:, bass.ds(h * IC_W, IC_W)],
                              in_=one_wh1[:, bass.ds(h * IC_W, IC_W)])
        nc.gpsimd.partition_broadcast(bcast_wh1h[h], wh1_rows[h][0:1, :])
        e1r = pool_small.tile([1, IC_W], F32, tag=f"e1r{h}", name=f"e1r{h}")
        nc.scalar.activation(e1r, wh1_rows[h], AF.Exp,
                             bias=bias4[0:1, 0:1], scale=1.0)
        f1r = pool_small.tile([1, IC_W], F32, tag=f"f1r{h}", name=f"f1r{h}")
        nc.scalar.activation(f1r, wh1_rows[h], AF.Exp,
                             bias=bias4[0:1, 0:1], scale=0.01)
        nc.gpsimd.partition_broadcast(bE1h[h], e1r[0:1, :])
        nc.gpsimd.partition_broadcast(bF1h[h], f1r[0:1, :])

    # wh2 reductions + e2/f2 cols + w2one row assembly (for route C),
    # interleaved with the whq2 and adjacency DMA streams so the first
    # A-quads of the main loop can start within a few us.
    w2d = [pool_dram.tile([P * 8], F32, tag=f"w2d{k}", name=f"w2d{k}")
           for k in range(8)]
    w2oneR = pool_const.tile([2, NJT * P], F32R)
    adjq_pre = {}
    CH = 1024
    w2st = [pool_small.tile([2, CH], F32, tag=f"w2st{k % 2}",
                            name=f"w2st{k % 2}") for k in range(2)]
    nc.vector.memset(w2st[0], 1.0)
    nc.vector.memset(w2st[1], 1.0)

    def dma_adj(ic, m):
        adjq = pool_adj.tile([P, 4, IC_W], FP8, tag="adjq", name="adjq")
        adj_eng = nc.scalar if m % 2 == 0 else nc.sync
        adj_eng.dma_start(
            adjq,
            adjT[bass.ds(m * 4 * P, 4 * P),
                 bass.ds(ic * IC_W, IC_W)].rearrange("(q p) i -> p q i", p=P))
        return adjq

    for g in range(8):
        dma_whq2(g)
        for m in (2 * g, 2 * g + 1):
            emit_wh2(m)
            adjq_pre[(0, m)] = dma_adj(0, m)
        nc.sync.dma_start(
            w2d[g][bass.ds(0, 4 * P)].rearrange("(t p) -> p t", p=P),
            wh2q[2 * g])
        nc.sync.dma_start(
            w2d[g][bass.ds(4 * P, 4 * P)].rearrange("(t p) -> p t", p=P),
            wh2q[2 * g + 1])
        nc.sync.dma_start(
            w2st[g % 2][0:1, :],
            w2d[g].rearrange("(o n) -> o n", o=1))
        nc.vector.tensor_copy(out=w2oneR[:, bass.ds(g * CH, CH)],
                              in_=w2st[g % 2])

    # ---- main loop --------------------------------------------------------
    for ic in range(NIC):
        psum_out = [
            pool_psum.tile([P, D], F32, tag=f"po{i}", name=f"po{i}")
            for i in range(ITPC)
        ]
        psum_rs = pool_psum.tile([P, 16], F32, tag="prs", name="prs")
        nc.vector.memset(psum_rs, 0.0)

        for m in range(NQ):
            route = ROUTES[ic][m]
            pmq = pool_pm.tile([P, 4, D], FP8, tag="pmq", name="pmq")
            adjq = adjq_pre.pop((ic, m), None)
            if adjq is None:
                adjq = dma_adj(ic, m)

            if route == 'C':
                for q in range(4):
                    jt = 4 * m + q
                    psc = pool_psc.tile([P, IC_W], F32, tag="psc", name="psc")
                    nc.tensor.matmul(
                        psc, lhsT=w2oneR[:, bass.ds(jt * P, P)],
                        rhs=one_wh1R[:, bass.ds(ic * IC_W, IC_W)],
                        start=True, stop=False, tile_position=(0, 0))
                    nc.tensor.matmul(psc, lhsT=eyeq, rhs=adjq[:, q, :],
                                     start=False, stop=True)
                    nc.scalar.activation(pmq[:, q, :], psc, AF.Exp)
            else:
                tq = pool_tq.tile([P, 4, IC_W], BF16, tag="tq", name="tq")
                if route == 'A':
                    for q in range(4):
                        nc.scalar.activation(
                            tq[:, q, :], bcast_wh1h[ic], AF.Exp,
                            bias=wh2q[m][:, q:q + 1], scale=1.0)
                else:
                    if m not in e2q:
                        e2c = pool_small.tile([P, 4], F32, tag=f"e2q{m}",
                                              name=f"e2q{m}")
                        nc.scalar.activation(e2c, wh2q[m], AF.Exp,
                                             bias=bias4[:, 0:1], scale=1.0)
                        f2c = pool_small.tile([P, 4], F32, tag=f"f2q{m}",
                                              name=f"f2q{m}")
                        nc.scalar.activation(f2c, wh2q[m], AF.Exp,
                                             bias=bias4[:, 0:1], scale=0.01)
                        e2q[m] = (e2c, f2c)
                    e2c, f2c = e2q[m]
                    for q in range(4):
                        nc.vector._custom_dve(
                            max_outer, out=tq[:, q, :],
                            in0=bE1h[ic], in1=bF1h[ic],
                            s0=e2c[:, q:q + 1],
                            s1=f2c[:, q:q + 1],
                            imm2=IMM2)
                geng = nc.vector if GATE_ENG[m] == 'v' else nc.gpsimd
                geng.scalar_tensor_tensor(
                    out=pmq, in0=adjq, scalar=-15.0, in1=tq,
                    op0=ALU.is_ge, op1=ALU.mult)

            first, last = (m == 0), (m == NQ - 1)
            for q in range(4):
                lhs1 = pmq[:, q, :]
                for i4 in range(ITPC):
                    lhsT = lhs1[:, bass.ds(i4 * P, P)].rearrange(
                        "p (o k) -> p o k", o=1).broadcast_to([P, 2, P])
                    nc.tensor.matmul(psum_out[i4], lhsT=lhsT,
                                     rhs=wh_pair(4 * m + q),
                                     start=(first and q == 0),
                                     stop=(last and q == 3),
                                     perf_mode=DR)
                    nc.tensor.matmul(psum_rs[:, 2 * i4:2 * i4 + 2], lhsT=lhsT,
                                     rhs=ones_hl[:, :, 0:2],
                                     start=False, stop=(last and q == 3),
                                     perf_mode=DR, skip_group_check=True)

        outq = pool_outs.tile([P, ITPC, D], BF16, tag=f"outq{ic}",
                              name=f"outq{ic}")
        for i4 in range(ITPC):
            recip = pool_small.tile([P, 1], F32, tag="recip", name="recip",
                                    bufs=2)
            nc.vector.reciprocal(recip, psum_rs[:, 2 * i4:2 * i4 + 1])
            nc.vector.tensor_scalar_mul(outq[:, i4, :], psum_out[i4], recip)
        nc.scalar.dma_start(
            out[bass.ds(ic * IC_W, IC_W), :].rearrange("(q p) d -> p q d", p=P),
            outq)


_CACHED = None


def build_nc():
    global _CACHED
    if _CACHED is not None:
        return _CACHED
    os.environ["BASS_ACT_ROOT_JSON_PATH"] = _make_fused_act_root()
    max_outer = register_max_outer()
    nc = bacc.Bacc("TRN2", target_bir_lowering=False, debug=False,
                   enable_asserts=False, num_devices=NCORES)
    adjT = nc.dram_tensor("adjT", [N, ROWS], FP8, kind="ExternalInput").ap()
    xw2 = nc.dram_tensor("xw2", [N, 2, D], FP8, kind="ExternalInput").ap()
    xloc = nc.dram_tensor("xloc", [ROWS, D], BF16, kind="ExternalInput").ap()
    a_t = nc.dram_tensor("a", [P, D + W2_W], F32, kind="ExternalInput").ap()
    eye8 = nc.dram_tensor("eye8", [P, P], FP8, kind="ExternalInput").ap()
    out = nc.dram_tensor("out", [ROWS, D], BF16, kind="ExternalOutput").ap()

    from contextlib import ExitStack
    with tile.TileContext(nc) as tc:
        with ExitStack() as ctx:
            _build_kernel(nc, tc, adjT, xw2, xloc, a_t, eye8, out, ctx,
                          max_outer)
    nc.compile()
    _CACHED = nc
    return nc


_PERM = None


def _d_perm(a):
    """Feature permutation: sort d by |a2| descending (host-side layout)."""
    global _PERM
    if _PERM is None:
        a2 = np.asarray(a, dtype=np.float32).reshape(-1)[D:]
        _PERM = np.argsort(-np.abs(a2), kind="stable")
    return _PERM


def make_in_maps(input, adj_matrix, a):
    perm = _d_perm(a)
    x = np.asarray(input, dtype=np.float32)[:, perm]
    a_np = np.asarray(a, dtype=np.float32).reshape(-1).copy()
    a1 = a_np[:D][perm]
    a2 = a_np[D:][perm]
    a_row = np.concatenate([a1, a2, a2[:LO_W]])
    a_rep = np.ascontiguousarray(
        np.broadcast_to(a_row[None, :], (P, D + W2_W)))

    x_hi = x.astype(ml_dtypes.float8_e4m3)
    x_lo = (x - x_hi.astype(np.float32)).astype(ml_dtypes.float8_e4m3)
    xw2 = np.ascontiguousarray(np.stack([x_hi, x_lo], axis=1))  # [N, 2, D]
    eye = np.eye(P, dtype=np.float32).astype(ml_dtypes.float8_e4m3)

    adj = np.asarray(adj_matrix)
    in_maps = []
    for c in range(NCORES):
        rows = slice(c * ROWS, (c + 1) * ROWS)
        adjT_c = np.ascontiguousarray(
            ((adj[rows, :].T.astype(np.float32) - 1.0) * 30.0)
            .astype(ml_dtypes.float8_e4m3))  # {edge: 0, non-edge: -30}
        in_maps.append({
            "adjT": adjT_c,
            "xw2": xw2,
            "xloc": np.ascontiguousarray(x[rows].astype(ml_dtypes.bfloat16)),
            "a": a_rep,
            "eye8": eye,
        })
    return in_maps


def kernel(input, adj_matrix, a, _trace=False, _tmpdir=None):
    nc = build_nc()
    in_maps = make_in_maps(input, adj_matrix, a)
    try:
        res = run_bass_kernel_spmd(nc, in_maps, core_ids=list(range(NCORES)),
                                   trace=_trace, tmpdir=_tmpdir)
    except ModuleNotFoundError:
        res = run_bass_kernel_spmd(nc, in_maps, core_ids=list(range(NCORES)))
    perm = _d_perm(a)
    inv = np.empty_like(perm)
    inv[perm] = np.arange(D)
    out = np.concatenate(
        [res.results[c]["out"].astype(np.float32)[:, inv]
         for c in range(NCORES)], axis=0)
    kernel._last_results = res
    return out


# revision 31
# speedup vs baseline: 1.1141x; 1.1141x over previous
"""GAT-style attentive layer on 8 TRN2 NeuronCores.

Math (per reference):
    Wh  = input                      [N, D]   (N=8192, D=512)
    Wh1 = Wh @ a[:D]                 [N, 1]
    Wh2 = Wh @ a[D:]                 [N, 1]
    e   = leaky_relu(Wh1 + Wh2.T, 0.01)
    e   = where(adj > 0, e, -9e15)
    att = softmax(e, axis=1)
    out = att @ Wh                   [N, D]

Sharding: row-shard the N x N attention across 8 cores (1024 rows each).
Per core, scores are produced directly in TRANSPOSED layout
pT[j, i] = exp(lrelu(Wh1[i] + Wh2[j])) * adj[i, j]  (j on partitions), so the
final matmul out[i,:] = sum_j pT[j,i] * Wh[j,:] can use pT tiles as the
stationary operand with no on-device transpose.  Softmax needs no
max-subtraction (|scores| <= ~6), and masked entries are exact zeros, so
out = (pT.T @ [Wh | 1]) then divide by the ones-column (row sums).

Host-side prep (data marshaling only): each core's adjacency slice is
delivered pre-transposed as fp8e4m3 ({0,1} exact) so the device DMA is
contiguous; all heavy compute (score gen, exp, mask, matmul, normalize)
runs on device.
"""

import numpy as np
import ml_dtypes

import concourse.bass as bass
import concourse.mybir as mybir
import concourse.tile as tile
from concourse import bacc
from concourse.bass_utils import run_bass_kernel_spmd

N = 8192          # nodes
D = 512           # feature dim
NCORES = 8
ROWS = N // NCORES  # 1024 output rows per core
P = 128
NJT = N // P      # 64 j-tiles per core
NIT = ROWS // P   # 8 i-tiles per core
IC_W = 512        # i-chunk width (PSUM-limited)
NIC = ROWS // IC_W  # 2 i-chunks
ITPC = IC_W // P  # 4 i-subtiles per chunk

import os
SIM_SAFE = os.environ.get("KERNEL_SIM_SAFE", "0") == "1"

AF = mybir.ActivationFunctionType
ALU = mybir.AluOpType
dt = mybir.dt
F32 = dt.float32
F32R = dt.float32r
BF16 = dt.bfloat16
FP8 = dt.float8e4


def _build_kernel(nc: bass.Bass, tc: tile.TileContext,
                  adjT: bass.AP, xw: bass.AP, xloc: bass.AP, a: bass.AP,
                  out: bass.AP, ctx):
    pool_const = ctx.enter_context(tc.tile_pool(name="const", bufs=1))
    pool_wh = ctx.enter_context(tc.tile_pool(name="wh", bufs=1))
    pool_adj = ctx.enter_context(tc.tile_pool(name="adj", bufs=4))
    pool_act = ctx.enter_context(tc.tile_pool(name="act", bufs=4))
    pool_pm = ctx.enter_context(tc.tile_pool(name="pm", bufs=6))
    pool_outs = ctx.enter_context(tc.tile_pool(name="outs", bufs=1))
    pool_small = ctx.enter_context(tc.tile_pool(name="small", bufs=1))
    pool_psum = ctx.enter_context(tc.tile_pool(name="psum", bufs=1, space="PSUM"))
    pool_dram = ctx.enter_context(tc.tile_pool(name="dram", bufs=1, space="DRAM"))

    # ---- constants / small prep -------------------------------------------
    # `a` arrives host-replicated across partitions: abc[p, :] = a[:, 0]
    abc = pool_const.tile([P, 2 * D], F32)
    nc.sync.dma_start(abc, a)
    bcast_a1 = abc[:, 0:D]
    bcast_a2 = abc[:, D:2 * D]

    warm = pool_const.tile([1, 2], F32)
    nc.vector.memset(warm, 0.0)
    nc.scalar.activation(warm, warm, AF.Exp)  # pull ACT_TABLE_LOAD to t~0

    ones_f32 = pool_const.tile([P, 2], F32)
    nc.vector.memset(ones_f32, 1.0)
    ones_col = pool_const.tile([P, 2], F32R)
    nc.vector.tensor_copy(out=ones_col, in_=ones_f32)

    # ---- Wh (= x) resident quads + per-quad Wh2 = x @ a2 columns ---------
    whq = []
    wh2_sb = pool_const.tile([P, NJT], F32)

    def wh_quad(m):
        t = pool_wh.tile([P, 4, D], F32R, tag=f"whq{m}", name=f"whq{m}")
        nc.sync.dma_start(
            t, xw[bass.ds(m * 4 * P, 4 * P), :].rearrange(
                "(q p) d -> p q d", p=P).bitcast(F32R))
        whq.append(t)
        for q in range(4):
            jt = 4 * m + q
            scr = pool_small.tile([P, D], F32, tag="g_scr", name="g_scr", bufs=2)
            nc.vector.scalar_tensor_tensor(
                out=scr, in0=t[:, q, :].bitcast(F32), scalar=0.0, in1=bcast_a2,
                op0=ALU.add, op1=ALU.mult,
                accum_out=wh2_sb[:, jt:jt + 1])

    # ---- Wh1 = xloc @ a1 for this core's 1024 rows — FIRST, since the whole
    # main loop gates on bcast_wh1 (score-tile input).  Done in two 512-row
    # halves so the ic=0 half of bcast_wh1 is ready ~7us in.
    wh1_rows = [pool_const.tile([1, IC_W], F32, tag=f"wh1r{h}", name=f"wh1r{h}")
                for h in range(NIC)]
    bcast_wh1h = [pool_const.tile([P, IC_W], F32, tag=f"bwh1{h}", name=f"bwh1{h}")
                  for h in range(NIC)]
    for h in range(NIC):
        wh1_half = pool_small.tile([P, ITPC], F32, tag=f"wh1h{h}", name=f"wh1h{h}")
        for q in range(4):
            xlt = pool_small.tile([P, D], F32, tag="xlt", name="xlt", bufs=3)
            nc.sync.dma_start(
                xlt, xloc[bass.ds((h * 4 + q) * P, P), :])
            scr = pool_small.tile([P, D], F32, tag="v_scr", name="v_scr")
            nc.vector.scalar_tensor_tensor(
                out=scr, in0=xlt, scalar=0.0, in1=bcast_a1,
                op0=ALU.add, op1=ALU.mult,
                accum_out=wh1_half[:, q:q + 1])
        # Flatten via DRAM roundtrip: row[t*128+k] = half[k, t]
        scr_d = pool_dram.tile([IC_W], F32, tag=f"wh1d{h}", name=f"wh1d{h}")
        nc.sync.dma_start(scr_d.rearrange("(t p) -> p t", p=P), wh1_half)
        nc.sync.dma_start(
            wh1_rows[h], scr_d.rearrange("(o n) -> o n", o=1))
        nc.gpsimd.partition_broadcast(bcast_wh1h[h], wh1_rows[h][0:1, :])

    # ---- remaining Wh quads + adj(ic=0) quads, interleaved so DMA arrival
    # order matches the j-loop's consumption order.
    adjq_pre = []

    def dma_adjq(m, ic):
        t = pool_adj.tile([P, 4, IC_W], FP8, tag="adjq", name="adjq")
        nc.sync.dma_start(
            t, adjT[bass.ds(m * 4 * P, 4 * P),
                    bass.ds(ic * IC_W, IC_W)].rearrange("(q p) i -> p q i", p=P))
        return t

    for m in range(NJT // 4):
        wh_quad(m)
        adjq_pre.append(dma_adjq(m, 0))

    # ---- main loop --------------------------------------------------------
    for ic in range(NIC):
        psum_out = [
            pool_psum.tile([P, D], F32, tag=f"po{i}", name=f"po{i}")
            for i in range(ITPC)
        ]
        psum_rs = [
            pool_psum.tile([P, 2], F32, tag=f"prs{i}", name=f"prs{i}")
            for i in range(ITPC)
        ]

        for jt in range(NJT):
            m, q = divmod(jt, 4)
            if q == 0:
                adjq = adjq_pre[m] if ic == 0 else dma_adjq(m, ic)

            p_t = pool_act.tile([P, IC_W], F32, tag="p_t", name="p_t")
            if SIM_SAFE:
                # CoreSim has no fused table: Identity score + DVE leaky-relu
                # + plain Exp (numerics-identical, slower).
                s_t = pool_act.tile([P, IC_W], F32, tag="s_t", name="s_t")
                nc.scalar.activation(
                    s_t, bcast_wh1h[ic], AF.Identity,
                    bias=wh2_sb[:, jt:jt + 1], scale=1.0)
                l_t = pool_act.tile([P, IC_W], F32, tag="l_t", name="l_t")
                nc.vector.scalar_tensor_tensor(
                    out=l_t, in0=s_t, scalar=0.01, in1=s_t,
                    op0=ALU.mult, op1=ALU.max)
                nc.scalar.activation(p_t, l_t, AF.Exp)
            else:
                # Patched act table: Exp's negative-x buckets hold
                # exp(0.01*x), so this one op is exp(leaky_relu(s)).
                nc.scalar.activation(
                    p_t, bcast_wh1h[ic], AF.Exp,
                    bias=wh2_sb[:, jt:jt + 1], scale=1.0)

            pm_t = pool_pm.tile([P, IC_W], F32R, tag="pm_t", name="pm_t")
            mask_eng = nc.gpsimd if (q >= 2) else nc.vector
            mask_eng.tensor_mul(out=pm_t, in0=p_t, in1=adjq[:, q, :])

            first, last = (jt == 0), (jt == NJT - 1)
            for i4 in range(ITPC):
                lhs = pm_t[:, bass.ds(i4 * P, P)]
                nc.tensor.matmul(psum_out[i4], lhsT=lhs,
                                 rhs=whq[m][:, q, :],
                                 start=first, stop=last)
                nc.tensor.matmul(psum_rs[i4], lhsT=lhs,
                                 rhs=ones_col,
                                 start=first, stop=last)

        outq = pool_outs.tile([P, ITPC, D], F32, tag="outq", name="outq")
        for i4 in range(ITPC):
            recip = pool_small.tile([P, 1], F32, tag="recip", name="recip")
            nc.vector.reciprocal(recip, psum_rs[i4][:, 0:1])
            nc.vector.tensor_scalar_mul(outq[:, i4, :], psum_out[i4], recip)
        nc.sync.dma_start(
            out[bass.ds(ic * IC_W, IC_W), :].rearrange("(q p) d -> p q d", p=P),
            outq)


_CACHED = None

_FUSED_ALPHA = 0.01


def _make_fused_act_root() -> str:
    """Copy the compiler's activation-table dir, patching exp's negative-x
    buckets from exp(x) to exp(_FUSED_ALPHA*x) splines (linear only - the
    function is nearly flat there).  Exp then computes exp(leaky_relu(x)) in
    a single ScalarE pass.  Returns path to the patched act_info.json."""
    import json
    import shutil
    import tempfile

    from neuronxcc.driver.Job import Job
    from neuronxcc.driver.jobs.support.FindActInfo import findActInfoFile

    src_root = os.path.dirname(findActInfoFile(Job.getPackageDir(), "gen3"))
    dst = tempfile.mkdtemp(prefix="act_root_fused_")
    for f in os.listdir(src_root):
        shutil.copy(os.path.join(src_root, f), os.path.join(dst, f))
    info = json.load(open(os.path.join(dst, "act_info.json")))
    for s in info["act_func_sets"]:
        if "exp" not in s["act"]:
            continue
        prof = json.load(open(os.path.join(dst, s["profile_json"])))
        order = sorted(prof["func_to_bkt_start_idx"].items(), key=lambda kv: kv[1])
        idx = [i for i, (k, _) in enumerate(order) if k == "exp"][0]
        lo = order[idx][1]
        hi = order[idx + 1][1] if idx + 1 < len(order) else prof["bkt_entry_cnt"]
        path = os.path.join(dst, s["bkt_bin"])
        bkt = np.fromfile(path, dtype=np.float32).reshape(-1, 8).copy()
        for b in range(lo, hi):
            d0, d1, _, _, x0 = bkt[b, :5]
            if not (d0 > 0 and abs(d1 - d0) <= 1e-3 * d0):
                continue  # saturation buckets (inf / 0)
            if x0 > 0:
                continue  # positive side: exp(x) unchanged
            g = np.float32(np.exp(_FUSED_ALPHA * np.float64(x0)))
            bkt[b, 0] = g
            bkt[b, 1] = np.float32(_FUSED_ALPHA * g)
            bkt[b, 2] = np.float32(0.0)  # cubic terms fault the engine
            bkt[b, 3] = np.float32(0.0)
        bkt.tofile(path)
    return os.path.join(dst, "act_info.json")


def build_nc():
    global _CACHED
    if _CACHED is not None:
        return _CACHED
    if not SIM_SAFE:
        # Always point the compiler at our patched tables: with the stock
        # tables this kernel's Exp op would silently drop the leaky-relu.
        os.environ["BASS_ACT_ROOT_JSON_PATH"] = _make_fused_act_root()
    nc = bacc.Bacc("TRN2", target_bir_lowering=False, debug=False,
                   enable_asserts=False, num_devices=NCORES)
    adjT = nc.dram_tensor("adjT", [N, ROWS], FP8, kind="ExternalInput").ap()
    xw = nc.dram_tensor("xw", [N, D], F32, kind="ExternalInput").ap()
    xloc = nc.dram_tensor("xloc", [ROWS, D], F32, kind="ExternalInput").ap()
    a_t = nc.dram_tensor("a", [P, 2 * D], F32, kind="ExternalInput").ap()
    out = nc.dram_tensor("out", [ROWS, D], F32, kind="ExternalOutput").ap()

    from contextlib import ExitStack
    with tile.TileContext(nc) as tc:
        with ExitStack() as ctx:
            _build_kernel(nc, tc, adjT, xw, xloc, a_t, out, ctx)
    nc.compile()
    _CACHED = nc
    return nc


def make_in_maps(input, adj_matrix, a):
    x = np.ascontiguousarray(np.asarray(input, dtype=np.float32))
    adj = np.asarray(adj_matrix)
    a_np = np.ascontiguousarray(
        np.broadcast_to(np.asarray(a, dtype=np.float32).reshape(1, -1), (P, 2 * D)))
    in_maps = []
    for c in range(NCORES):
        rows = slice(c * ROWS, (c + 1) * ROWS)
        adjT_c = np.ascontiguousarray(
            adj[rows, :].T.astype(ml_dtypes.float8_e4m3))  # {0,1} exact in fp8
        in_maps.append({
            "adjT": adjT_c,
            "xw": x,
            "xloc": np.ascontiguousarray(x[rows]),
            "a": a_np,
        })
    return in_maps


def kernel(input, adj_matrix, a, _trace=False, _tmpdir=None):
    nc = build_nc()
    in_maps = make_in_maps(input, adj_matrix, a)
    try:
        res = run_bass_kernel_spmd(nc, in_maps, core_ids=list(range(NCORES)),
                                   trace=_trace, tmpdir=_tmpdir)
    except ModuleNotFoundError:
        # NTFF profiling hooks absent in this container; run untraced.
        res = run_bass_kernel_spmd(nc, in_maps, core_ids=list(range(NCORES)))
    out = np.concatenate([res.results[c]["out"] for c in range(NCORES)], axis=0)
    kernel._last_results = res
    return out

